# revision 48
# baseline (speedup 1.0000x reference)
"""Trainium2 Bass kernel for nn_DirectionalEncoding.

Computation (per batch element b, sharded batch-parallel over 8 cores):
  dp_norm = dp / max(||dp||_2 over xyz, 1e-12)          # (3, N, K)
  theta   = vec_norm @ dp_norm                          # (M=32, N, K)
  tmax    = max over K                                  # (32, N)
  h       = gelu(bn(w1 @ tmax))                         # (128, N)
  f       = w2 @ h + b2                                 # (256, N)

Layout strategy on-chip (all matmuls in float32r, ~1.3e-4 rel err):
  * dp loaded as (128 part = point-group, 3, 128, 16) -- contiguous DMA,
    front pipelined in 8 n_in-blocks (DMA/normalize/transpose overlap)
  * normalize pointwise: squares+sums+r-scale on VectorE, rsqrt as
    exp(-0.5*ln(sq)) on ScalarE (direct Rsqrt activation is banned)
  * PE transposes (vs f32r identity) re-pack normalized dp to
    B = (48 part = (d,k), N); PSUM->SBUF copies split ScalarE/VectorE
  * stage-1: 4 matmuls per 512-col chunk; lhsT packs vec_norm with a
    k-selection so theta lands as (kk*32+m, n) on all 128 partitions
  * max over 16 k: one VectorE reduce_max over the 4-tile PSUM group,
    then fold 128->64 partitions via PE identity-shift + VectorE max,
    and 64->32 via max(a,b) = (a+b)/2 + |a-b|/2 (PE sum/diff matmul,
    ScalarE Abs, PE accumulate) to keep VectorE off the critical path
  * gemm1 with BN scale folded into weights, exact-erf Gelu(+BN bias)
    on ScalarE, gemm2, per-partition b2 bias on output copies
"""

import sys
from contextlib import ExitStack

sys.path.insert(0, "/opt/trn_rl_repo")

import numpy as np

import concourse.bass as bass
import concourse.mybir as mybir
import concourse.tile as tile
from concourse import bacc
from concourse.bass_utils import run_bass_kernel_spmd

F32 = mybir.dt.float32
F32R = mybir.dt.float32r
AF = mybir.ActivationFunctionType

B, N, K, M = 8, 16384, 16, 32
H, O = 128, 256
P = 128            # partitions
NIN = N // P       # 128 points per partition
CH = 512           # n-chunk size
NCH = N // CH      # 32 chunks
EPS_BN = 1e-5

_CACHE = {}


def _patch_act_tables():
    """Constrain the act-table-set picker to two sets that jointly cover all
    activation functions this kernel uses (ln/exp/copy front, gelu/identity/
    copy main loop), so the greedy picker cannot thrash table loads.
    Set ids (dict order) are preserved."""
    import concourse.hw_specs as hw_specs

    if getattr(bacc, "_act_tables_patched", False):
        return
    orig = bacc.get_activation_tables
    keep = {"natural_log_exp_and_others", "gelu_and_others"}

    def patched(arch):
        tables = orig(arch)
        out = {}
        for name, fns in tables.items():
            out[name] = fns if name in keep else set()
        return out

    bacc.get_activation_tables = patched
    bacc._act_tables_patched = True


def _build_program():
    _patch_act_tables()
    nc = bacc.Bacc("TRN2", target_bir_lowering=False, debug=False)

    dp_d = nc.dram_tensor("dp", [3, N, K], F32, kind="ExternalInput").ap()
    wsel_d = nc.dram_tensor("wsel", [48, 4 * P], F32R, kind="ExternalInput").ap()
    w1t_d = nc.dram_tensor("w1t", [M, H], F32R, kind="ExternalInput").ap()
    w2t_d = nc.dram_tensor("w2t", [H, O], F32R, kind="ExternalInput").ap()
    bbn_d = nc.dram_tensor("bbn", [H, 1], F32, kind="ExternalInput").ap()
    b2h_d = nc.dram_tensor("b2h", [P, 2], F32, kind="ExternalInput").ap()
    id_d = nc.dram_tensor("ident", [P, P], F32, kind="ExternalInput").ap()
    lst_d = nc.dram_tensor("lst", [64, 64], F32R, kind="ExternalInput").ap()
    f_d = nc.dram_tensor("f", [O, N], F32, kind="ExternalOutput").ap()

    with tile.TileContext(nc) as tc:
        with (
            tc.tile_pool(name="const", bufs=1) as cpool,
            tc.tile_pool(name="big", bufs=1) as big,
            tc.tile_pool(name="tree", bufs=4) as tree,
            tc.tile_pool(name="tmax", bufs=4) as tmaxp,
            tc.tile_pool(name="zp", bufs=3) as zp,
            tc.tile_pool(name="ob", bufs=3) as obp,
        ):
            ident = cpool.tile([P, P], F32)
            nc.sync.dma_start(ident[:], id_d[:])
            identr = cpool.tile([P, P], F32R)
            nc.sync.dma_start(identr[:], id_d.bitcast(F32R))
            wsel = cpool.tile([48, 4 * P], F32R)
            nc.sync.dma_start(wsel[:], wsel_d[:])
            w1t = cpool.tile([M, H], F32R)
            nc.sync.dma_start(w1t[:], w1t_d[:])
            w2t = cpool.tile([H, O], F32R)
            nc.sync.dma_start(w2t[:], w2t_d[:])
            bbn = cpool.tile([H, 1], F32)
            nc.sync.dma_start(bbn[:], bbn_d[:])
            b2h = cpool.tile([P, 2], F32)
            nc.sync.dma_start(b2h[:], b2h_d[:])
            lst = cpool.tile([64, 64], F32R)
            nc.sync.dma_start(lst[:], lst_d[:])

            # ---- front pipeline over quarters of n_in:
            # DMA slice -> squares (ACT) -> adds (GpSimd) -> rsqrt (ACT)
            # -> muls (GpSimd) -> PE transposes -> B copies (ACT)
            X = big.tile([P, 3, NIN, K], F32)
            dpr = dp_d.rearrange("d (p q) k -> p d q k", p=P)
            s1 = big.tile([P, NIN, K], F32)
            s2 = big.tile([P, NIN, K], F32)
            sq = big.tile([P, NIN, K], F32)
            Y = big.tile([P, NIN, 3, K], F32R)
            Yv = Y[:].rearrange("p q d k -> p q (d k)")
            Bt = big.tile([48, N], F32R)
            Bv = Bt[:].rearrange("c (pp q) -> c pp q", q=NIN)
            NQ = 8
            QW = NIN // NQ
            with tc.tile_pool(name="ptp", bufs=4, space="PSUM") as ptp:
                for q in range(NQ):
                    qs = slice(q * QW, (q + 1) * QW)
                    nc.sync.dma_start(X[:, :, qs, :], dpr[:, :, qs, :])
                    nc.vector.tensor_mul(sq[:, qs, :], X[:, 0, qs, :], X[:, 0, qs, :])
                    nc.vector.tensor_mul(s1[:, qs, :], X[:, 1, qs, :], X[:, 1, qs, :])
                    nc.vector.tensor_mul(s2[:, qs, :], X[:, 2, qs, :], X[:, 2, qs, :])
                    nc.vector.tensor_add(sq[:, qs, :], sq[:, qs, :], s1[:, qs, :])
                    nc.vector.tensor_add(sq[:, qs, :], sq[:, qs, :], s2[:, qs, :])
                    # rsqrt = exp(-0.5 * ln(sq))   (ScalarE Rsqrt is banned)
                    nc.scalar.activation(s1[:, qs, :], sq[:, qs, :], AF.Ln)
                    nc.scalar.activation(s2[:, qs, :], s1[:, qs, :], AF.Exp,
                                         scale=-0.5)
                    for d in range(3):
                        nc.vector.tensor_mul(Y[:, qs, d, :], X[:, d, qs, :],
                                             s2[:, qs, :])
                    for t in range(q * QW // 4, (q + 1) * QW // 4):
                        pt = ptp.tile([48, 4, P], F32R)
                        for j in range(4):
                            nc.tensor.transpose(
                                pt[:, j, :], Yv[:, 4 * t + j, :],
                                identr[:],
                            )
                        if t % 2 == 0:
                            nc.scalar.copy(
                                Bv[:, :, 4 * t : 4 * t + 4],
                                pt[:].rearrange("c j pp -> c pp j"),
                            )
                        else:
                            nc.vector.tensor_copy(
                                Bv[:, :, 4 * t : 4 * t + 4],
                                pt[:].rearrange("c j pp -> c pp j"),
                            )

            # ---- per-chunk: stage1 matmuls, max tree, gemm1, gelu, gemm2, out
            fv = f_d.rearrange("(h o) n -> o h n", h=2)
            _ps = ExitStack()
            thp = _ps.enter_context(tc.tile_pool(name="thp", bufs=1, space="PSUM"))
            shp = _ps.enter_context(tc.tile_pool(name="shp", bufs=2, space="PSUM"))
            ph2 = _ps.enter_context(tc.tile_pool(name="ph2", bufs=1, space="PSUM"))
            for c in range(NCH):
                cs = slice(c * CH, (c + 1) * CH)
                th = thp.tile([P, 4, CH], F32)
                for g in range(4):
                    nc.tensor.matmul(
                        th[:, g, :],
                        wsel[:, g * P : (g + 1) * P],
                        Bt[:, cs],
                        start=True,
                        stop=True,
                    )
                # fold g (4 tiles) in one pool_max over innermost AP dim
                t03 = tree.tile([P, CH], F32R, tag="t03")
                thg = th[:].rearrange("p g n -> p n g")
                nc.vector.reduce_max(t03[:], thg, axis=mybir.AxisListType.X)
                # kk folds: PE partition-shift matmuls + DVE maxes
                p1 = shp.tile([64, CH], F32, tag="sh")
                nc.tensor.matmul(p1[:], identr[64:128, 64:128], t03[64:128, :],
                                 start=True, stop=True)
                tu = tree.tile([64, CH], F32R, tag="tu")
                nc.vector.tensor_max(tu[:], t03[0:64, :], p1[:])
                # fold2: max(a,b) = (a+b)/2 + |a-b|/2 with zero DVE ops
                pst2 = shp.tile([64, CH], F32, tag="sh")
                nc.tensor.matmul(pst2[:], lst[:], tu[:], start=True, stop=False)
                u2 = tree.tile([M, CH], F32R, tag="u2")
                nc.scalar.activation(u2[:], pst2[M : 2 * M, :], AF.Abs)
                nc.tensor.matmul(pst2[0:M, :], identr[0:M, 0:M], u2[:],
                                 start=False, stop=True)
                tm = tmaxp.tile([M, CH], F32R, tag="tm")
                nc.scalar.copy(tm[:], pst2[0:M, :])

                ph = shp.tile([H, CH], F32, tag="sh")
                nc.tensor.matmul(ph[:], w1t[:], tm[:], start=True, stop=True)
                z = zp.tile([H, CH], F32R)
                nc.scalar.activation(z[:], ph[:], AF.Gelu, bias=bbn[:, 0:1])

                pf = ph2.tile([P, 2, CH], F32)
                nc.tensor.matmul(
                    pf[:, 0, :], w2t[:, 0:P], z[:], start=True, stop=True
                )
                nc.tensor.matmul(
                    pf[:, 1, :], w2t[:, P : 2 * P], z[:], start=True, stop=True
                )
                ob = obp.tile([P, 2, CH], F32)
                nc.scalar.add(ob[:, 0, :], pf[:, 0, :], b2h[:, 0:1])
                nc.vector.tensor_scalar_add(ob[:, 1, :], pf[:, 1, :], b2h[:, 1:2])
                nc.sync.dma_start(fv[:, :, cs], ob[:])
            _ps.close()

    nc.compile()
    return nc


def _host_weights(dir_vectors, w1, bn_gamma, bn_beta, bn_mean, bn_var, w2, b2):
    f32 = np.float32
    dv = dir_vectors.astype(f32)
    nrm = np.sqrt(np.sum(dv * dv, axis=1, dtype=f32).astype(f32)).astype(f32)
    vecn = dv / np.maximum(nrm, f32(1e-12))[:, None]

    wsel = np.zeros((48, 4 * P), dtype=f32)
    for g in range(4):
        for kk in range(4):
            k = g * 4 + kk
            for d in range(3):
                wsel[d * 16 + k, g * P + kk * M : g * P + (kk + 1) * M] = vecn[:, d]

    a_bn = (bn_gamma.astype(f32) / np.sqrt(bn_var.astype(f32) + f32(EPS_BN))).astype(f32)
    w1a = a_bn[:, None] * w1.astype(f32)
    w1t = np.ascontiguousarray(w1a.T)                      # (32, 128)
    bbn = (bn_beta.astype(f32) - bn_mean.astype(f32) * a_bn).reshape(H, 1)
    w2t = np.ascontiguousarray(w2.astype(f32).T)           # (128, 256)
    b2h = np.ascontiguousarray(b2.astype(f32).reshape(2, P).T)  # (128, 2)
    ident = np.eye(P, dtype=f32)
    lst = np.zeros((64, 64), dtype=f32)
    for j in range(32):
        lst[j, j] = 0.5
        lst[32 + j, j] = 0.5
        lst[j, 32 + j] = 0.5
        lst[32 + j, 32 + j] = -0.5
    return {
        "lst": lst,
        "wsel": wsel,
        "w1t": w1t,
        "w2t": w2t,
        "bbn": np.ascontiguousarray(bbn),
        "b2h": b2h,
        "ident": ident,
    }


def kernel(dp, f0, idx, dir_vectors, w1, bn_gamma, bn_beta, bn_mean, bn_var,
           w2, b2, **_unused):
    del f0, idx  # unused by the reference computation
    if "nc" not in _CACHE:
        _CACHE["nc"] = _build_program()
    nc = _CACHE["nc"]

    dir_vectors, w1, bn_gamma, bn_beta, bn_mean, bn_var, w2, b2 = (
        np.asarray(x, dtype=np.float32)
        for x in (dir_vectors, w1, bn_gamma, bn_beta, bn_mean, bn_var, w2, b2)
    )
    wts = _host_weights(dir_vectors, w1, bn_gamma, bn_beta, bn_mean, bn_var,
                        w2, b2)
    dp = np.asarray(dp, dtype=np.float32)
    in_maps = [
        {"dp": np.ascontiguousarray(dp[b]), **wts} for b in range(B)
    ]
    res = run_bass_kernel_spmd(nc, in_maps, list(range(B)))
    out = np.stack([res.results[b]["f"] for b in range(B)], axis=0)
    return out.astype(np.float32)


if __name__ == "__main__":
    rng = np.random.default_rng(0)
    ins = {
        "dp": rng.standard_normal((B, 3, N, K), dtype=np.float32),
        "f0": rng.standard_normal((B, 3, N), dtype=np.float32),
        "idx": rng.integers(0, N, size=(B, N, K)).astype(np.int64),
        "dir_vectors": rng.standard_normal((M, 3), dtype=np.float32),
        "w1": (rng.standard_normal((H, M), dtype=np.float32) / np.sqrt(M)).astype(np.float32),
        "bn_gamma": np.ones(H, np.float32),
        "bn_beta": np.zeros(H, np.float32),
        "bn_mean": (0.1 * rng.standard_normal(H)).astype(np.float32),
        "bn_var": rng.uniform(0.5, 1.5, H).astype(np.float32),
        "w2": (rng.standard_normal((O, H), dtype=np.float32) / np.sqrt(H)).astype(np.float32),
        "b2": np.zeros(O, np.float32),
    }
    out = kernel(**ins)
    print("out", out.shape, out.dtype, float(np.abs(out).mean()))


# revision 50
# speedup vs baseline: 1.0056x; 1.0056x over previous
"""Trainium2 Bass kernel for nn_DirectionalEncoding.

Computation (per batch element b, sharded batch-parallel over 8 cores):
  dp_norm = dp / max(||dp||_2 over xyz, 1e-12)          # (3, N, K)
  theta   = vec_norm @ dp_norm                          # (M=32, N, K)
  tmax    = max over K                                  # (32, N)
  h       = gelu(bn(w1 @ tmax))                         # (128, N)
  f       = w2 @ h + b2                                 # (256, N)

Layout strategy on-chip (all matmuls in float32r, ~1.3e-4 rel err):
  * dp loaded as (128 part = point-group, 3, 128, 16) -- contiguous DMA,
    front pipelined in 8 n_in-blocks (DMA/normalize/transpose overlap)
  * normalize pointwise: squares+sums+r-scale on VectorE, rsqrt as
    exp(-0.5*ln(sq)) on ScalarE (direct Rsqrt activation is banned)
  * PE transposes (vs f32r identity) re-pack normalized dp to
    B = (48 part = (d,k), N); PSUM->SBUF copies split ScalarE/VectorE
  * stage-1: 4 matmuls per 512-col chunk; lhsT packs vec_norm with a
    k-selection so theta lands as (kk*32+m, n) on all 128 partitions
  * max over 16 k: one VectorE reduce_max over the 4-tile PSUM group,
    then fold 128->64 partitions via PE identity-shift + VectorE max,
    and 64->32 via max(a,b) = (a+b)/2 + |a-b|/2 (PE sum/diff matmul,
    ScalarE Abs, PE accumulate) to keep VectorE off the critical path
  * gemm1 with BN scale folded into weights, exact-erf Gelu(+BN bias)
    on ScalarE, gemm2, per-partition b2 bias on output copies
"""

import sys
from contextlib import ExitStack

sys.path.insert(0, "/opt/trn_rl_repo")

import numpy as np

import concourse.bass as bass
import concourse.mybir as mybir
import concourse.tile as tile
from concourse import bacc
from concourse.bass_utils import run_bass_kernel_spmd

F32 = mybir.dt.float32
F32R = mybir.dt.float32r
AF = mybir.ActivationFunctionType

B, N, K, M = 8, 16384, 16, 32
H, O = 128, 256
P = 128            # partitions
NIN = N // P       # 128 points per partition
CH = 512           # n-chunk size
NCH = N // CH      # 32 chunks
EPS_BN = 1e-5

_CACHE = {}


def _patch_act_tables():
    """Constrain the act-table-set picker to two sets that jointly cover all
    activation functions this kernel uses (ln/exp/copy front, gelu/identity/
    copy main loop), so the greedy picker cannot thrash table loads.
    Set ids (dict order) are preserved."""
    import concourse.hw_specs as hw_specs

    if getattr(bacc, "_act_tables_patched", False):
        return
    orig = bacc.get_activation_tables
    keep = {"natural_log_exp_and_others", "gelu_and_others"}

    def patched(arch):
        tables = orig(arch)
        out = {}
        for name, fns in tables.items():
            out[name] = fns if name in keep else set()
        return out

    bacc.get_activation_tables = patched
    bacc._act_tables_patched = True


def _build_program():
    _patch_act_tables()
    nc = bacc.Bacc("TRN2", target_bir_lowering=False, debug=False)

    dp_d = nc.dram_tensor("dp", [3, N, K], F32, kind="ExternalInput").ap()
    wsel_d = nc.dram_tensor("wsel", [48, 4 * P], F32R, kind="ExternalInput").ap()
    w1t_d = nc.dram_tensor("w1t", [M, H], F32R, kind="ExternalInput").ap()
    w2t_d = nc.dram_tensor("w2t", [H, O], F32R, kind="ExternalInput").ap()
    bbn_d = nc.dram_tensor("bbn", [H, 1], F32, kind="ExternalInput").ap()
    b2h_d = nc.dram_tensor("b2h", [P, 2], F32, kind="ExternalInput").ap()
    id_d = nc.dram_tensor("ident", [P, P], F32, kind="ExternalInput").ap()
    lst_d = nc.dram_tensor("lst", [64, 64], F32R, kind="ExternalInput").ap()
    f_d = nc.dram_tensor("f", [O, N], F32, kind="ExternalOutput").ap()

    with tile.TileContext(nc) as tc:
        with (
            tc.tile_pool(name="const", bufs=1) as cpool,
            tc.tile_pool(name="big", bufs=1) as big,
            tc.tile_pool(name="tree", bufs=4) as tree,
            tc.tile_pool(name="tmax", bufs=4) as tmaxp,
            tc.tile_pool(name="zp", bufs=3) as zp,
            tc.tile_pool(name="ob", bufs=3) as obp,
        ):
            identr = cpool.tile([P, P], F32R)
            nc.sync.dma_start(identr[:], id_d.bitcast(F32R))
            wsel = cpool.tile([48, 4 * P], F32R)
            nc.sync.dma_start(wsel[:], wsel_d[:])
            w1t = cpool.tile([M, H], F32R)
            nc.sync.dma_start(w1t[:], w1t_d[:])
            w2t = cpool.tile([H, O], F32R)
            nc.sync.dma_start(w2t[:], w2t_d[:])
            bbn = cpool.tile([H, 1], F32)
            nc.sync.dma_start(bbn[:], bbn_d[:])
            b2h = cpool.tile([P, 2], F32)
            nc.sync.dma_start(b2h[:], b2h_d[:])
            lst = cpool.tile([64, 64], F32R)
            nc.sync.dma_start(lst[:], lst_d[:])

            # ---- front pipeline over quarters of n_in:
            # DMA slice -> squares (ACT) -> adds (GpSimd) -> rsqrt (ACT)
            # -> muls (GpSimd) -> PE transposes -> B copies (ACT)
            X = big.tile([P, 3, NIN, K], F32)
            dpr = dp_d.rearrange("d (p q) k -> p d q k", p=P)
            s1 = big.tile([P, NIN, K], F32)
            s2 = big.tile([P, NIN, K], F32)
            sq = big.tile([P, NIN, K], F32)
            Y = big.tile([P, NIN, 3, K], F32R)
            Yv = Y[:].rearrange("p q d k -> p q (d k)")
            Bt = big.tile([48, N], F32R)
            Bv = Bt[:].rearrange("c (pp q) -> c pp q", q=NIN)
            NQ = 8
            QW = NIN // NQ
            with tc.tile_pool(name="ptp", bufs=4, space="PSUM") as ptp:
                for q in range(NQ):
                    qs = slice(q * QW, (q + 1) * QW)
                    nc.sync.dma_start(X[:, :, qs, :], dpr[:, :, qs, :])
                    nc.vector.tensor_mul(sq[:, qs, :], X[:, 0, qs, :], X[:, 0, qs, :])
                    nc.vector.tensor_mul(s1[:, qs, :], X[:, 1, qs, :], X[:, 1, qs, :])
                    nc.vector.tensor_mul(s2[:, qs, :], X[:, 2, qs, :], X[:, 2, qs, :])
                    nc.vector.tensor_add(sq[:, qs, :], sq[:, qs, :], s1[:, qs, :])
                    nc.vector.tensor_add(sq[:, qs, :], sq[:, qs, :], s2[:, qs, :])
                    # rsqrt = exp(-0.5 * ln(sq))   (ScalarE Rsqrt is banned)
                    nc.scalar.activation(s1[:, qs, :], sq[:, qs, :], AF.Ln)
                    nc.scalar.activation(s2[:, qs, :], s1[:, qs, :], AF.Exp,
                                         scale=-0.5)
                    for d in range(3):
                        nc.vector.tensor_mul(Y[:, qs, d, :], X[:, d, qs, :],
                                             s2[:, qs, :])
                    for t in range(q * QW // 4, (q + 1) * QW // 4):
                        pt = ptp.tile([48, 4, P], F32R)
                        for j in range(4):
                            nc.tensor.transpose(
                                pt[:, j, :], Yv[:, 4 * t + j, :],
                                identr[:],
                            )
                        if t % 2 == 0:
                            nc.scalar.copy(
                                Bv[:, :, 4 * t : 4 * t + 4],
                                pt[:].rearrange("c j pp -> c pp j"),
                            )
                        else:
                            nc.vector.tensor_copy(
                                Bv[:, :, 4 * t : 4 * t + 4],
                                pt[:].rearrange("c j pp -> c pp j"),
                            )

            # ---- per-chunk: stage1 matmuls, max tree, gemm1, gelu, gemm2, out
            fv = f_d.rearrange("(h o) n -> o h n", h=2)
            _ps = ExitStack()
            thp = _ps.enter_context(tc.tile_pool(name="thp", bufs=1, space="PSUM"))
            shp = _ps.enter_context(tc.tile_pool(name="shp", bufs=2, space="PSUM"))
            ph2 = _ps.enter_context(tc.tile_pool(name="ph2", bufs=1, space="PSUM"))
            for c in range(NCH):
                cs = slice(c * CH, (c + 1) * CH)
                th = thp.tile([P, 4, CH], F32)
                for g in range(4):
                    nc.tensor.matmul(
                        th[:, g, :],
                        wsel[:, g * P : (g + 1) * P],
                        Bt[:, cs],
                        start=True,
                        stop=True,
                    )
                # fold g (4 tiles) in one pool_max over innermost AP dim
                t03 = tree.tile([P, CH], F32R, tag="t03")
                thg = th[:].rearrange("p g n -> p n g")
                nc.vector.reduce_max(t03[:], thg, axis=mybir.AxisListType.X)
                # kk folds: PE partition-shift matmuls + DVE maxes
                p1 = shp.tile([64, CH], F32, tag="sh")
                nc.tensor.matmul(p1[:], identr[64:128, 64:128], t03[64:128, :],
                                 start=True, stop=True)
                tu = tree.tile([64, CH], F32R, tag="tu")
                nc.vector.tensor_max(tu[:], t03[0:64, :], p1[:])
                # fold2: max(a,b) = (a+b)/2 + |a-b|/2 with zero DVE ops
                pst2 = shp.tile([64, CH], F32, tag="sh")
                nc.tensor.matmul(pst2[:], lst[:], tu[:], start=True, stop=False)
                u2 = tree.tile([M, CH], F32R, tag="u2")
                nc.scalar.activation(u2[:], pst2[M : 2 * M, :], AF.Abs)
                nc.tensor.matmul(pst2[0:M, :], identr[0:M, 0:M], u2[:],
                                 start=False, stop=True)
                tm = tmaxp.tile([M, CH], F32R, tag="tm")
                nc.scalar.copy(tm[:], pst2[0:M, :])

                ph = shp.tile([H, CH], F32, tag="sh")
                nc.tensor.matmul(ph[:], w1t[:], tm[:], start=True, stop=True)
                z = zp.tile([H, CH], F32R)
                nc.scalar.activation(z[:], ph[:], AF.Gelu, bias=bbn[:, 0:1])

                pf = ph2.tile([P, 2, CH], F32)
                nc.tensor.matmul(
                    pf[:, 0, :], w2t[:, 0:P], z[:], start=True, stop=True
                )
                nc.tensor.matmul(
                    pf[:, 1, :], w2t[:, P : 2 * P], z[:], start=True, stop=True
                )
                ob = obp.tile([P, 2, CH], F32)
                nc.scalar.add(ob[:, 0, :], pf[:, 0, :], b2h[:, 0:1])
                nc.vector.tensor_scalar_add(ob[:, 1, :], pf[:, 1, :], b2h[:, 1:2])
                nc.sync.dma_start(fv[:, :, cs], ob[:])
            _ps.close()

    nc.compile()
    return nc


def _host_weights(dir_vectors, w1, bn_gamma, bn_beta, bn_mean, bn_var, w2, b2):
    f32 = np.float32
    dv = dir_vectors.astype(f32)
    nrm = np.sqrt(np.sum(dv * dv, axis=1, dtype=f32).astype(f32)).astype(f32)
    vecn = dv / np.maximum(nrm, f32(1e-12))[:, None]

    wsel = np.zeros((48, 4 * P), dtype=f32)
    for g in range(4):
        for kk in range(4):
            k = g * 4 + kk
            for d in range(3):
                wsel[d * 16 + k, g * P + kk * M : g * P + (kk + 1) * M] = vecn[:, d]

    a_bn = (bn_gamma.astype(f32) / np.sqrt(bn_var.astype(f32) + f32(EPS_BN))).astype(f32)
    w1a = a_bn[:, None] * w1.astype(f32)
    w1t = np.ascontiguousarray(w1a.T)                      # (32, 128)
    bbn = (bn_beta.astype(f32) - bn_mean.astype(f32) * a_bn).reshape(H, 1)
    w2t = np.ascontiguousarray(w2.astype(f32).T)           # (128, 256)
    b2h = np.ascontiguousarray(b2.astype(f32).reshape(2, P).T)  # (128, 2)
    ident = np.eye(P, dtype=f32)
    lst = np.zeros((64, 64), dtype=f32)
    for j in range(32):
        lst[j, j] = 0.5
        lst[32 + j, j] = 0.5
        lst[j, 32 + j] = 0.5
        lst[32 + j, 32 + j] = -0.5
    return {
        "lst": lst,
        "wsel": wsel,
        "w1t": w1t,
        "w2t": w2t,
        "bbn": np.ascontiguousarray(bbn),
        "b2h": b2h,
        "ident": ident,
    }


def kernel(dp, f0, idx, dir_vectors, w1, bn_gamma, bn_beta, bn_mean, bn_var,
           w2, b2, **_unused):
    del f0, idx  # unused by the reference computation
    if "nc" not in _CACHE:
        _CACHE["nc"] = _build_program()
    nc = _CACHE["nc"]

    dir_vectors, w1, bn_gamma, bn_beta, bn_mean, bn_var, w2, b2 = (
        np.asarray(x, dtype=np.float32)
        for x in (dir_vectors, w1, bn_gamma, bn_beta, bn_mean, bn_var, w2, b2)
    )
    wts = _host_weights(dir_vectors, w1, bn_gamma, bn_beta, bn_mean, bn_var,
                        w2, b2)
    dp = np.asarray(dp, dtype=np.float32)
    in_maps = [
        {"dp": np.ascontiguousarray(dp[b]), **wts} for b in range(B)
    ]
    res = run_bass_kernel_spmd(nc, in_maps, list(range(B)))
    out = np.stack([res.results[b]["f"] for b in range(B)], axis=0)
    return out.astype(np.float32)


if __name__ == "__main__":
    rng = np.random.default_rng(0)
    ins = {
        "dp": rng.standard_normal((B, 3, N, K), dtype=np.float32),
        "f0": rng.standard_normal((B, 3, N), dtype=np.float32),
        "idx": rng.integers(0, N, size=(B, N, K)).astype(np.int64),
        "dir_vectors": rng.standard_normal((M, 3), dtype=np.float32),
        "w1": (rng.standard_normal((H, M), dtype=np.float32) / np.sqrt(M)).astype(np.float32),
        "bn_gamma": np.ones(H, np.float32),
        "bn_beta": np.zeros(H, np.float32),
        "bn_mean": (0.1 * rng.standard_normal(H)).astype(np.float32),
        "bn_var": rng.uniform(0.5, 1.5, H).astype(np.float32),
        "w2": (rng.standard_normal((O, H), dtype=np.float32) / np.sqrt(H)).astype(np.float32),
        "b2": np.zeros(O, np.float32),
    }
    out = kernel(**ins)
    print("out", out.shape, out.dtype, float(np.abs(out).mean()))


# revision 56
# speedup vs baseline: 1.0190x; 1.0134x over previous
"""Trainium2 Bass kernel for nn_DirectionalEncoding.

Computation (per batch element b, sharded batch-parallel over 8 cores):
  dp_norm = dp / max(||dp||_2 over xyz, 1e-12)          # (3, N, K)
  theta   = vec_norm @ dp_norm                          # (M=32, N, K)
  tmax    = max over K                                  # (32, N)
  h       = gelu(bn(w1 @ tmax))                         # (128, N)
  f       = w2 @ h + b2                                 # (256, N)

Layout strategy on-chip (all matmuls in float32r, ~1.3e-4 rel err):
  * dp loaded as (128 part = point-group, 3, 128, 16) -- contiguous DMA,
    front pipelined in 8 n_in-blocks (DMA/normalize/transpose overlap)
  * normalize pointwise: squares+sums+r-scale on VectorE, rsqrt as
    exp(-0.5*ln(sq)) on ScalarE (direct Rsqrt activation is banned)
  * PE transposes (vs f32r identity) re-pack normalized dp to
    B = (48 part = (d,k), N); PSUM->SBUF copies split ScalarE/VectorE
  * stage-1: 4 matmuls per 512-col chunk; lhsT packs vec_norm with a
    k-selection so theta lands as (kk*32+m, n) on all 128 partitions
  * max over 16 k: one VectorE reduce_max over the 4-tile PSUM group,
    then fold 128->64 partitions via PE identity-shift + VectorE max,
    and 64->32 via max(a,b) = (a+b)/2 + |a-b|/2 (PE sum/diff matmul,
    ScalarE Abs, PE accumulate) to keep VectorE off the critical path
  * gemm1 with BN scale folded into weights, exact-erf Gelu(+BN bias)
    on ScalarE, gemm2, per-partition b2 bias on output copies
"""

import sys
from contextlib import ExitStack

sys.path.insert(0, "/opt/trn_rl_repo")

import numpy as np

import concourse.bass as bass
import concourse.mybir as mybir
import concourse.tile as tile
from concourse import bacc
from concourse.bass_utils import run_bass_kernel_spmd

F32 = mybir.dt.float32
F32R = mybir.dt.float32r
AF = mybir.ActivationFunctionType

B, N, K, M = 8, 16384, 16, 32
H, O = 128, 256
P = 128            # partitions
NIN = N // P       # 128 points per partition
CH = 512           # n-chunk size
NCH = N // CH      # 32 chunks
EPS_BN = 1e-5

_CACHE = {}


def _patch_act_tables():
    """Constrain the act-table-set picker to two sets that jointly cover all
    activation functions this kernel uses (ln/exp/copy front, gelu/identity/
    copy main loop), so the greedy picker cannot thrash table loads.
    Set ids (dict order) are preserved."""
    import concourse.hw_specs as hw_specs

    if getattr(bacc, "_act_tables_patched", False):
        return
    orig = bacc.get_activation_tables
    keep = {"natural_log_exp_and_others", "gelu_and_others"}

    def patched(arch):
        tables = orig(arch)
        out = {}
        for name, fns in tables.items():
            out[name] = fns if name in keep else set()
        return out

    bacc.get_activation_tables = patched
    bacc._act_tables_patched = True


def _build_program():
    _patch_act_tables()
    nc = bacc.Bacc("TRN2", target_bir_lowering=False, debug=False)

    dp_d = nc.dram_tensor("dp", [3, N, K], F32, kind="ExternalInput").ap()
    # all f32r weights in one blob: [ident | wsel | w1t | w2t | lst]
    wb_d = nc.dram_tensor("wblob", [P, 1088], F32R, kind="ExternalInput").ap()
    bb_d = nc.dram_tensor("bblob", [P, 3], F32, kind="ExternalInput").ap()
    f_d = nc.dram_tensor("f", [O, N], F32, kind="ExternalOutput").ap()

    with tile.TileContext(nc) as tc:
        with (
            tc.tile_pool(name="const", bufs=1) as cpool,
            tc.tile_pool(name="big", bufs=1) as big,
            tc.tile_pool(name="tree", bufs=4) as tree,
            tc.tile_pool(name="tmax", bufs=4) as tmaxp,
            tc.tile_pool(name="zp", bufs=3) as zp,
            tc.tile_pool(name="ob", bufs=3) as obp,
        ):
            wb = cpool.tile([P, 1088], F32R)
            nc.sync.dma_start(wb[:], wb_d[:])
            bb = cpool.tile([P, 3], F32)
            nc.sync.dma_start(bb[:], bb_d[:])
            identr = wb[:, 0:128]
            wsel = wb[0:48, 128:640]
            w1t = wb[0:32, 640:768]
            w2t = wb[:, 768:1024]
            lst = wb[0:64, 1024:1088]
            bbn = bb[:, 0:1]
            b2h = bb[:, 1:3]

            # ---- front pipeline over quarters of n_in:
            # DMA slice -> squares (ACT) -> adds (GpSimd) -> rsqrt (ACT)
            # -> muls (GpSimd) -> PE transposes -> B copies (ACT)
            X = big.tile([P, 3, NIN, K], F32)
            dpr = dp_d.rearrange("d (p q) k -> p d q k", p=P)
            s1 = big.tile([P, NIN, K], F32)
            s2 = big.tile([P, NIN, K], F32)
            sq = big.tile([P, NIN, K], F32)
            Y = big.tile([P, NIN, 3, K], F32R)
            Yv = Y[:].rearrange("p q d k -> p q (d k)")
            Bt = big.tile([48, N], F32R)
            Bv = Bt[:].rearrange("c (pp q) -> c pp q", q=NIN)
            NQ = 8
            QW = NIN // NQ
            with tc.tile_pool(name="ptp", bufs=4, space="PSUM") as ptp:
                for q in range(NQ):
                    qs = slice(q * QW, (q + 1) * QW)
                    nc.sync.dma_start(X[:, :, qs, :], dpr[:, :, qs, :])
                    nc.vector.tensor_mul(sq[:, qs, :], X[:, 0, qs, :], X[:, 0, qs, :])
                    nc.vector.tensor_mul(s1[:, qs, :], X[:, 1, qs, :], X[:, 1, qs, :])
                    nc.vector.tensor_mul(s2[:, qs, :], X[:, 2, qs, :], X[:, 2, qs, :])
                    nc.vector.tensor_add(sq[:, qs, :], sq[:, qs, :], s1[:, qs, :])
                    nc.vector.tensor_add(sq[:, qs, :], sq[:, qs, :], s2[:, qs, :])
                    # rsqrt = exp(-0.5 * ln(sq))   (ScalarE Rsqrt is banned)
                    nc.scalar.activation(s1[:, qs, :], sq[:, qs, :], AF.Ln)
                    nc.scalar.activation(s2[:, qs, :], s1[:, qs, :], AF.Exp,
                                         scale=-0.5)
                    for d in range(3):
                        nc.vector.tensor_mul(Y[:, qs, d, :], X[:, d, qs, :],
                                             s2[:, qs, :])
                    for t in range(q * QW // 4, (q + 1) * QW // 4):
                        pt = ptp.tile([48, 4, P], F32R)
                        for j in range(4):
                            nc.tensor.transpose(
                                pt[:, j, :], Yv[:, 4 * t + j, :],
                                identr,
                            )
                        if t % 2 == 0:
                            nc.scalar.copy(
                                Bv[:, :, 4 * t : 4 * t + 4],
                                pt[:].rearrange("c j pp -> c pp j"),
                            )
                        else:
                            nc.vector.tensor_copy(
                                Bv[:, :, 4 * t : 4 * t + 4],
                                pt[:].rearrange("c j pp -> c pp j"),
                            )

            # ---- per-chunk: stage1 matmuls, max tree, gemm1, gelu, gemm2, out
            fv = f_d.rearrange("(h o) n -> o h n", h=2)
            _ps = ExitStack()
            thp = _ps.enter_context(tc.tile_pool(name="thp", bufs=1, space="PSUM"))
            shp = _ps.enter_context(tc.tile_pool(name="shp", bufs=2, space="PSUM"))
            ph2 = _ps.enter_context(tc.tile_pool(name="ph2", bufs=1, space="PSUM"))
            for c in range(NCH):
                cs = slice(c * CH, (c + 1) * CH)
                th = thp.tile([P, 4, CH], F32)
                for g in range(4):
                    nc.tensor.matmul(
                        th[:, g, :],
                        wsel[:, g * P : (g + 1) * P],
                        Bt[:, cs],
                        start=True,
                        stop=True,
                    )
                # fold g (4 tiles) in one pool_max over innermost AP dim
                t03 = tree.tile([P, CH], F32R, tag="t03")
                thg = th[:].rearrange("p g n -> p n g")
                nc.vector.reduce_max(t03[:], thg, axis=mybir.AxisListType.X)
                # kk folds: PE partition-shift matmuls + DVE maxes
                p1 = shp.tile([64, CH], F32, tag="sh")
                nc.tensor.matmul(p1[:], identr[64:128, 64:128], t03[64:128, :],
                                 start=True, stop=True)
                tu = tree.tile([64, CH], F32R, tag="tu")
                nc.vector.tensor_max(tu[:], t03[0:64, :], p1[:])
                # fold2: max(a,b) = (a+b)/2 + |a-b|/2 with zero DVE ops
                pst2 = shp.tile([64, CH], F32, tag="sh")
                nc.tensor.matmul(pst2[:], lst, tu[:], start=True, stop=False)
                u2 = tree.tile([M, CH], F32R, tag="u2")
                nc.scalar.activation(u2[:], pst2[M : 2 * M, :], AF.Abs)
                nc.tensor.matmul(pst2[0:M, :], identr[0:M, 0:M], u2[:],
                                 start=False, stop=True)
                tm = tmaxp.tile([M, CH], F32R, tag="tm")
                nc.scalar.copy(tm[:], pst2[0:M, :])

                ph = shp.tile([H, CH], F32, tag="sh")
                nc.tensor.matmul(ph[:], w1t, tm[:], start=True, stop=True)
                z = zp.tile([H, CH], F32R)
                nc.scalar.activation(z[:], ph[:], AF.Gelu, bias=bbn)

                pf = ph2.tile([P, 2, CH], F32)
                nc.tensor.matmul(
                    pf[:, 0, :], w2t[:, 0:P], z[:], start=True, stop=True
                )
                nc.tensor.matmul(
                    pf[:, 1, :], w2t[:, P : 2 * P], z[:], start=True, stop=True
                )
                ob = obp.tile([P, 2, CH], F32)
                nc.scalar.add(ob[:, 0, :], pf[:, 0, :], b2h[:, 0:1])
                nc.vector.tensor_scalar_add(ob[:, 1, :], pf[:, 1, :], b2h[:, 1:2])
                nc.sync.dma_start(fv[:, :, cs], ob[:])
            _ps.close()

    nc.compile()
    return nc


def _host_weights(dir_vectors, w1, bn_gamma, bn_beta, bn_mean, bn_var, w2, b2):
    f32 = np.float32
    dv = dir_vectors.astype(f32)
    nrm = np.sqrt(np.sum(dv * dv, axis=1, dtype=f32).astype(f32)).astype(f32)
    vecn = dv / np.maximum(nrm, f32(1e-12))[:, None]

    wsel = np.zeros((48, 4 * P), dtype=f32)
    for g in range(4):
        for kk in range(4):
            k = g * 4 + kk
            for d in range(3):
                wsel[d * 16 + k, g * P + kk * M : g * P + (kk + 1) * M] = vecn[:, d]

    a_bn = (bn_gamma.astype(f32) / np.sqrt(bn_var.astype(f32) + f32(EPS_BN))).astype(f32)
    w1a = a_bn[:, None] * w1.astype(f32)
    w1t = np.ascontiguousarray(w1a.T)                      # (32, 128)
    bbn = (bn_beta.astype(f32) - bn_mean.astype(f32) * a_bn).reshape(H, 1)
    w2t = np.ascontiguousarray(w2.astype(f32).T)           # (128, 256)
    b2h = np.ascontiguousarray(b2.astype(f32).reshape(2, P).T)  # (128, 2)
    ident = np.eye(P, dtype=f32)
    lst = np.zeros((64, 64), dtype=f32)
    for j in range(32):
        lst[j, j] = 0.5
        lst[32 + j, j] = 0.5
        lst[j, 32 + j] = 0.5
        lst[32 + j, 32 + j] = -0.5
    wblob = np.zeros((P, 1088), dtype=f32)
    wblob[:, 0:128] = ident
    wblob[0:48, 128:640] = wsel
    wblob[0:32, 640:768] = w1t
    wblob[:, 768:1024] = w2t
    wblob[0:64, 1024:1088] = lst
    bblob = np.zeros((P, 3), dtype=f32)
    bblob[:, 0:1] = bbn
    bblob[:, 1:3] = b2h
    return {"wblob": wblob, "bblob": bblob}


def kernel(dp, f0, idx, dir_vectors, w1, bn_gamma, bn_beta, bn_mean, bn_var,
           w2, b2, **_unused):
    del f0, idx  # unused by the reference computation
    if "nc" not in _CACHE:
        _CACHE["nc"] = _build_program()
    nc = _CACHE["nc"]

    dir_vectors, w1, bn_gamma, bn_beta, bn_mean, bn_var, w2, b2 = (
        np.asarray(x, dtype=np.float32)
        for x in (dir_vectors, w1, bn_gamma, bn_beta, bn_mean, bn_var, w2, b2)
    )
    wts = _host_weights(dir_vectors, w1, bn_gamma, bn_beta, bn_mean, bn_var,
                        w2, b2)
    dp = np.asarray(dp, dtype=np.float32)
    in_maps = [
        {"dp": np.ascontiguousarray(dp[b]), **wts} for b in range(B)
    ]
    res = run_bass_kernel_spmd(nc, in_maps, list(range(B)))
    out = np.stack([res.results[b]["f"] for b in range(B)], axis=0)
    return out.astype(np.float32)


if __name__ == "__main__":
    rng = np.random.default_rng(0)
    ins = {
        "dp": rng.standard_normal((B, 3, N, K), dtype=np.float32),
        "f0": rng.standard_normal((B, 3, N), dtype=np.float32),
        "idx": rng.integers(0, N, size=(B, N, K)).astype(np.int64),
        "dir_vectors": rng.standard_normal((M, 3), dtype=np.float32),
        "w1": (rng.standard_normal((H, M), dtype=np.float32) / np.sqrt(M)).astype(np.float32),
        "bn_gamma": np.ones(H, np.float32),
        "bn_beta": np.zeros(H, np.float32),
        "bn_mean": (0.1 * rng.standard_normal(H)).astype(np.float32),
        "bn_var": rng.uniform(0.5, 1.5, H).astype(np.float32),
        "w2": (rng.standard_normal((O, H), dtype=np.float32) / np.sqrt(H)).astype(np.float32),
        "b2": np.zeros(O, np.float32),
    }
    out = kernel(**ins)
    print("out", out.shape, out.dtype, float(np.abs(out).mean()))
